# revision 12
# baseline (speedup 1.0000x reference)
"""Trainium2 Bass kernel for nn_EvolvableSNN (T=512, B=8, N=4096, LIF SNN).

Strategy
--------
The LIF dynamics with these parameters are sub-threshold: the membrane
potential equilibrium is ~tau_mem*tau_syn*cur ~= 1e-4 * cur, four orders of
magnitude below threshold=1.0, so no neuron ever spikes and the recurrent
feedback term is identically zero.  With zero feedback the scan is a LINEAR
time-invariant filter of the feedforward drive:

    ff    = input[:, :, :512] @ W_in                      # [T, B, N]
    mem_t = DT^2 * sum_{s<=t} g(t-s) * ff_s               # per (b, n)
    g(d)  = (b^(d+1) - a^(d+1)) / (b - a),  a = 1-DT/tau_syn, b = 1-DT/tau_mem
    spikes_t = (mem_t >= threshold)

so mem = GT.T @_time (x @ W_in) -- two chained dense matmuls, fully parallel
across (batch, neuron).  Validity is guarded by a rigorous norm bound
computed on the host:

    max|mem| <= DT^2 * sum_d g(d) * max_row||x_row||_2 * max_col||W_col||_2

(~2e-3 for the target inputs, vs threshold 1.0).  If the bound does not
clear min(threshold) by a wide margin -- or the device reports any spike --
we fall back to an exact sequential numpy port of the reference.  The first
spike of the no-feedback system coincides with the first spike of the true
system, so "no spikes under linearization" exactly implies correctness.

Numerics: matmul operands are rounded to bf16 (accumulation in fp32 PSUM).
The guard includes the bf16 error allowance; spike values {0,1} are exact
in the bf16 output, which the host casts back to fp32.

Sharding: (NBG batch-groups x NNG neuron-column-groups) grid over 8 cores.
Each core runs the same program on its input slice; no collectives.
"""

import numpy as np
import ml_dtypes

import concourse.bass as bass
import concourse.mybir as mybir
import concourse.tile as tile
from concourse import bacc, bass_utils

# Problem constants (hardcoded per harness contract).
T, B, N = 512, 8, 4096
IN = 512          # INPUT_SIZE
DT = 0.001
P = 128           # SBUF partitions
NCORES = 8

# Core grid: NBG batch-groups x NNG neuron-groups (NBG * NNG == NCORES).
NBG, NNG = 4, 2
NB_LOC = B // NBG          # batches per core
NW = N // NNG              # neuron columns per core
KI = IN // P               # contraction tiles over input dim (4)
KT = T // P                # tiles over time dim (4)
NCH = NW // 512            # 512-wide n chunks per core
F32 = mybir.dt.float32
BF16 = mybir.dt.bfloat16
NPBF16 = ml_dtypes.bfloat16

MARGIN = 0.1               # abs margin to min(threshold) for the fast path

_compiled = None           # cached compiled Bass module


def _filter_taps(alpha: float, beta: float) -> np.ndarray:
    """g(d) * DT^2 for d = 0..T-1 (float64)."""
    d = np.arange(T, dtype=np.float64)
    if abs(beta - alpha) > 1e-12:
        g = (beta ** (d + 1) - alpha ** (d + 1)) / (beta - alpha)
    else:
        g = (d + 1) * alpha**d
    return g * DT * DT


def _build_gt(alpha: float, beta: float) -> np.ndarray:
    """GT[s, t] = DT^2 * g(t - s) for s <= t else 0 (upper-triangular)."""
    g = _filter_taps(alpha, beta)
    s = np.arange(T)
    diff = s[None, :] - s[:, None]  # diff[s, t] = t - s
    gt = np.where(diff >= 0, g[np.clip(diff, 0, T - 1)], 0.0)
    return gt.astype(np.float32)


def _build_device():
    """Compile the per-core Tile kernel once; returns the Bass module.

    Input layouts are pre-packed on the host so every DMA is one large
    fully-contiguous transfer:
      x  [NB_LOC*P, KT*IN]  row (b*P + p), col (k*IN + i) = x_b[k*P+p, i]
      w  [P, KI*NW]         row p, col (k*NW + n)         = W_in[k*P+p, n]
      gt [P, KT*T]          row p, col (k*T + t)          = GT[k*P+p, t]
    """
    nc = bacc.Bacc(
        "TRN2", target_bir_lowering=False, debug=False, num_devices=NCORES
    )
    x = nc.dram_tensor("x", [NB_LOC * P, KT * IN], BF16, kind="ExternalInput").ap()
    w = nc.dram_tensor("w", [P, KI * NW], BF16, kind="ExternalInput").ap()
    gt = nc.dram_tensor("gt", [P, KT * T], BF16, kind="ExternalInput").ap()
    th = nc.dram_tensor("th", [P, NW], F32, kind="ExternalInput").ap()
    spk = nc.dram_tensor("spk", [NB_LOC * T, NW], BF16, kind="ExternalOutput").ap()

    with tile.TileContext(nc) as tc:
        with (
            tc.tile_pool(name="const", bufs=1) as cpool,
            tc.tile_pool(name="xin", bufs=2) as xpool,
            tc.tile_pool(name="xg", bufs=2) as xgpool,
            tc.tile_pool(name="sout", bufs=4) as spool,
            tc.tile_pool(name="ps1", bufs=2, space="PSUM") as ps1,
            tc.tile_pool(name="ps2", bufs=4, space="PSUM") as ps2,
        ):
            # load order: stage-1 operands first so PE starts ASAP; w is
            # chunked by j (column order (j, k)) so stage-2 j=0 can start
            # as soon as its 0.5MB lands.
            # spread input loads over independent DMA paths (sync HWDGE ring,
            # scalar/ACT HWDGE ring, gpsimd SWDGE) so transfers overlap
            gt_sb = cpool.tile([P, KT * T], BF16, tag="gt")
            nc.sync.dma_start(gt_sb, gt)
            x_sb = {}
            x_sb[0] = xpool.tile([P, KT * IN], BF16, tag="x", name="x0")
            nc.scalar.dma_start(x_sb[0], x[0:P, :])
            w_sb = cpool.tile([P, KI * NW], BF16, tag="w")
            JW = KI * 512  # columns per j-chunk in the (j, k) packed layout
            for j in range(NCH):
                nc.sync.dma_start(
                    w_sb[:, j * JW : (j + 1) * JW], w[:, j * JW : (j + 1) * JW]
                )
            th_sb = cpool.tile([P, NW], F32, tag="th")
            nc.scalar.dma_start(th_sb, th)
            for b in range(1, NB_LOC):
                x_sb[b] = xpool.tile([P, KT * IN], BF16, tag="x", name=f"x{b}")
                nc.gpsimd.dma_start(x_sb[b], x[b * P : (b + 1) * P, :])

            for b in range(NB_LOC):
                # stage 1: xgT[i, t] = sum_s x_b[s, i] * GT[s, t]
                # GT[s, t] == 0 for t < s: s-tile k only feeds t >= 128k.
                xg_sb = []
                for m in range(KI):
                    p1 = ps1.tile([P, T], F32, tag="p1")
                    for k in range(KT):
                        nc.tensor.matmul(
                            p1[:, k * P :],
                            x_sb[b][:, k * IN + m * P : k * IN + (m + 1) * P],
                            gt_sb[:, k * T + k * P : (k + 1) * T],
                            start=(k == 0),
                            stop=(k == KT - 1),
                            skip_group_check=True,
                        )
                    xgt = xgpool.tile([P, T], BF16, tag=f"xg{m}")
                    nc.scalar.copy(xgt, p1)
                    xg_sb.append(xgt)
                # stage 2: mem[t, n] = sum_i xgT[i, t] * W_in[i, n]
                for mt in range(KT):
                    s_sb = spool.tile([P, NW], BF16, tag="s")
                    for j in range(NCH):
                        p2 = ps2.tile([P, 512], F32, tag="p2")
                        for k in range(KI):
                            nc.tensor.matmul(
                                p2,
                                xg_sb[k][:, mt * P : (mt + 1) * P],
                                w_sb[:, j * JW + k * 512 : j * JW + (k + 1) * 512],
                                start=(k == 0),
                                stop=(k == KI - 1),
                            )
                        nc.vector.tensor_tensor(
                            s_sb[:, j * 512 : (j + 1) * 512],
                            p2,
                            th_sb[:, j * 512 : (j + 1) * 512],
                            op=mybir.AluOpType.is_ge,
                        )
                    eng = nc.gpsimd if mt % 2 == 0 else nc.sync
                    eng.dma_start(
                        spk[b * T + mt * P : b * T + (mt + 1) * P, :], s_sb
                    )
    nc.compile()
    return nc


def _run_device(x_bm, W_in, gt_np, threshold, trace=False):
    """Run the SPMD kernel; returns (spikes [T,B,N] f32, results obj).

    x_bm: [B*T, IN] float32 batch-major rows; W_in: [IN, N]; gt_np [T, T].
    """
    global _compiled
    if _compiled is None:
        _compiled = _build_device()
    nc = _compiled
    # pack to device layouts (see _build_device docstring)
    x_bf = x_bm.astype(NPBF16)  # [B*T, IN]
    w_bf = W_in.astype(NPBF16)  # [IN, N]
    gt_bf = gt_np.astype(NPBF16)  # [T, T]
    gt_pack = np.ascontiguousarray(
        gt_bf.reshape(KT, P, T).transpose(1, 0, 2).reshape(P, KT * T)
    )
    # x: per batch b: [T, IN] -> [P, KT*IN]
    x_pack_all = np.ascontiguousarray(
        x_bf.reshape(B, KT, P, IN).transpose(0, 2, 1, 3).reshape(B * P, KT * IN)
    )
    in_maps = []
    for c in range(NCORES):
        bg, ng = divmod(c, NNG)
        xs = np.ascontiguousarray(
            x_pack_all[bg * NB_LOC * P : (bg + 1) * NB_LOC * P]
        )
        # [IN, NW] -> [P, (j, k, 512)]: row p, col j*KI*512 + k*512 + nn
        wc = np.ascontiguousarray(
            w_bf[:, ng * NW : (ng + 1) * NW]
            .reshape(KI, P, NCH, 512)
            .transpose(1, 2, 0, 3)
            .reshape(P, KI * NW)
        )
        thc = np.ascontiguousarray(
            np.broadcast_to(threshold[ng * NW : (ng + 1) * NW], (P, NW))
        )
        in_maps.append({"x": xs, "w": wc, "gt": gt_pack, "th": thc})
    res = bass_utils.run_bass_kernel_spmd(
        nc, in_maps, core_ids=list(range(NCORES)), trace=trace
    )
    out = np.zeros((B, T, N), dtype=np.float32)
    for c in range(NCORES):
        bg, ng = divmod(c, NNG)
        s = res.results[c]["spk"].astype(np.float32).reshape(NB_LOC, T, NW)
        out[bg * NB_LOC : (bg + 1) * NB_LOC, :, ng * NW : (ng + 1) * NW] = s
    return out.transpose(1, 0, 2), res


def _fallback(input_signal, weights, tau_mem, tau_syn, threshold):
    """Exact sequential port of the reference (numpy float32)."""
    x = np.asarray(input_signal, dtype=np.float32)
    w = np.asarray(weights, dtype=np.float32)
    W_in, W_rec = w[:IN], w[IN:]
    Tt, Bb, Nn = x.shape
    ff = np.einsum("tbi,in->tbn", x[:, :, :IN], W_in).astype(np.float32)
    syn = np.zeros((Bb, Nn), np.float32)
    mem = np.zeros((Bb, Nn), np.float32)
    fb = np.zeros((Bb, Nn), np.float32)
    out = np.zeros((Tt, Bb, Nn), np.float32)
    for t in range(Tt):
        cur = ff[t] + fb
        syn = syn + (-syn / tau_syn + cur) * np.float32(DT)
        mem = mem + (-mem / tau_mem + syn) * np.float32(DT)
        spikes = (mem >= threshold).astype(np.float32)
        mem = mem * (1.0 - spikes)
        rec = spikes[:, IN:] @ W_rec
        rec[:, :IN] = 0.0
        fb = rec
        out[t] = spikes
    return out


def kernel(input_signal, weights, tau_mem, tau_syn, threshold, _trace=False):
    input_signal = np.asarray(input_signal)
    weights = np.asarray(weights)
    tau_mem = np.asarray(tau_mem)
    tau_syn = np.asarray(tau_syn)
    threshold = np.asarray(threshold)

    ok_shape = (
        input_signal.shape == (T, B, N)
        and weights.shape == (N, N)
        and np.all(tau_mem == tau_mem.flat[0])
        and np.all(tau_syn == tau_syn.flat[0])
        and np.all(np.isfinite(input_signal))
        and np.all(np.isfinite(weights[:IN]))
    )
    if not ok_shape:
        return _fallback(input_signal, weights, tau_mem, tau_syn, threshold)

    alpha = 1.0 - DT / float(tau_syn.flat[0])
    beta = 1.0 - DT / float(tau_mem.flat[0])
    if not (0.0 <= alpha < 1.0 and 0.0 <= beta < 1.0):
        # numerically unstable / nonstandard regime: be safe
        return _fallback(input_signal, weights, tau_mem, tau_syn, threshold)

    # Rigorous sub-threshold bound (exact arithmetic + bf16 allowance):
    # |mem[t,b,n]| <= sum_d g(d)DT^2 * max|ff|,
    # |ff[s,b,n]| <= max_row ||x_row||_2 * max_col ||W_col||_2.
    x_in = input_signal[:, :, :IN].astype(np.float64)
    W_in64 = weights[:IN].astype(np.float64)
    max_row = np.sqrt((x_in * x_in).sum(axis=2).max())
    max_col = np.sqrt((W_in64 * W_in64).sum(axis=0).max())
    gsum = _filter_taps(alpha, beta).sum()
    mem_bound = gsum * max_row * max_col
    # 5% headroom for bf16 rounding of operands + fp32 accumulation error
    safe = mem_bound * 1.05 < float(threshold.min()) - MARGIN
    if not safe:
        return _fallback(input_signal, weights, tau_mem, tau_syn, threshold)

    gt_np = _build_gt(alpha, beta)
    # batch-major rows: row (b*T + t) = input_signal[t, b, :IN]
    x_bm = np.ascontiguousarray(
        input_signal[:, :, :IN].transpose(1, 0, 2).reshape(B * T, IN)
    ).astype(np.float32, copy=False)
    W_in = np.ascontiguousarray(weights[:IN]).astype(np.float32, copy=False)

    spikes, _ = _run_device(
        x_bm, W_in, gt_np, threshold.astype(np.float32), trace=_trace
    )
    if spikes.any():
        # bound said sub-threshold yet device saw spikes: distrust, recompute
        return _fallback(input_signal, weights, tau_mem, tau_syn, threshold)
    return spikes


# revision 15
# speedup vs baseline: 1.1222x; 1.1222x over previous
"""Trainium2 Bass kernel for nn_EvolvableSNN (T=512, B=8, N=4096, LIF SNN).

Strategy
--------
The LIF dynamics with these parameters are sub-threshold: the membrane
potential equilibrium is ~tau_mem*tau_syn*cur ~= 1e-4 * cur, four orders of
magnitude below threshold=1.0, so no neuron ever spikes and the recurrent
feedback term is identically zero.  With zero feedback the scan is a LINEAR
time-invariant filter of the feedforward drive:

    ff    = input[:, :, :512] @ W_in                      # [T, B, N]
    mem_t = DT^2 * sum_{s<=t} g(t-s) * ff_s               # per (b, n)
    g(d)  = (b^(d+1) - a^(d+1)) / (b - a),  a = 1-DT/tau_syn, b = 1-DT/tau_mem
    spikes_t = (mem_t >= threshold)

so mem = GT.T @_time (x @ W_in) -- two chained dense matmuls, fully parallel
across (batch, neuron).  Validity is guarded by a rigorous norm bound
computed on the host:

    max|mem| <= DT^2 * sum_d g(d) * max_row||x_row||_2 * max_col||W_col||_2

(~2e-3 for the target inputs, vs threshold 1.0).  If the bound (inflated by
the mixed-precision error allowance, see below) does not clear
min(threshold) by a wide margin -- or the device reports any spike -- we
fall back to an exact sequential numpy port of the reference.  The first
spike of the no-feedback system coincides with the first spike of the true
system, so "no spikes under linearization" exactly implies correctness.

Numerics: stage 1 (time filter) runs in bf16 operands with fp32 PSUM
accumulation; stage 2 (x W_in product) runs in fp8-e4m3 DoubleRow (2x PE
throughput) with power-of-two scale factors sx (on xg, applied by the
Scalar-engine PSUM->SBUF copy) and sw (folded into W on the host).  The
threshold is pre-scaled by sx*sw on the host, so the comparison
(mem*sx*sw >= th*sx*sw) is exactly monotone-equivalent.  Spike values {0,1}
are exact in the bf16 output, which the host casts back to fp32.

Sharding: (NBG batch-groups x NNG neuron-column-groups) grid over 8 cores.
Each core runs the same program on its input slice; no collectives.
"""

import math

import numpy as np
import ml_dtypes

import concourse.bass as bass
import concourse.mybir as mybir
import concourse.tile as tile
from concourse import bacc, bass_utils

# Problem constants (hardcoded per harness contract).
T, B, N = 512, 8, 4096
IN = 512          # INPUT_SIZE
DT = 0.001
P = 128           # SBUF partitions
NCORES = 8

# Core grid: NBG batch-groups x NNG neuron-groups (NBG * NNG == NCORES).
NBG, NNG = 4, 2
NB_LOC = B // NBG          # batches per core
NW = N // NNG              # neuron columns per core
KI = IN // P               # contraction tiles over input dim (4)
KP = KI // 2               # DoubleRow contraction pair-tiles (2)
KT = T // P                # tiles over time dim (4)
NCH = NW // 512            # 512-wide n chunks per core
F32 = mybir.dt.float32
BF16 = mybir.dt.bfloat16
FP8 = mybir.dt.float8e4
NPBF16 = ml_dtypes.bfloat16
NPFP8 = ml_dtypes.float8_e4m3

MARGIN = 0.1               # abs margin to min(threshold) for the fast path

_compiled = None           # cached compiled Bass module
LAST_RES = None            # last device results (for external profiling)


def _filter_taps(alpha: float, beta: float) -> np.ndarray:
    """g(d) * DT^2 for d = 0..T-1 (float64)."""
    d = np.arange(T, dtype=np.float64)
    if abs(beta - alpha) > 1e-12:
        g = (beta ** (d + 1) - alpha ** (d + 1)) / (beta - alpha)
    else:
        g = (d + 1) * alpha**d
    return g * DT * DT


def _build_gt(alpha: float, beta: float) -> np.ndarray:
    """GT[s, t] = DT^2 * g(t - s) for s <= t else 0 (upper-triangular)."""
    g = _filter_taps(alpha, beta)
    s = np.arange(T)
    diff = s[None, :] - s[:, None]  # diff[s, t] = t - s
    gt = np.where(diff >= 0, g[np.clip(diff, 0, T - 1)], 0.0)
    return gt.astype(np.float32)


def _build_device():
    """Compile the per-core Tile kernel once; returns the Bass module.

    Input layouts are pre-packed on the host so every DMA is one large
    fully-contiguous transfer:
      x  [NB_LOC*P, KT*IN]     row (b*P + p), col (k*IN + i) = x_b[k*P+p, i]
      w  [P, NCH, KP, 2, 512]  fp8, w[p, j, kp, i2, n]
                               = W_in[(2kp+i2)*128+p, j*512+n] * sw
      gt [P, KT*T]             row p, col (k*T + t) = GT[k*P+p, t]
      th [P, NW]               threshold * sx * sw, replicated rows
      sc [P, 1]                sx (runtime scale for the stage-1 copy)
    """
    nc = bacc.Bacc(
        "TRN2", target_bir_lowering=False, debug=False, num_devices=NCORES
    )
    x = nc.dram_tensor("x", [NB_LOC * P, KT * IN], BF16, kind="ExternalInput").ap()
    w = nc.dram_tensor("w", [P, NCH, KP, 2, 512], FP8, kind="ExternalInput").ap()
    gt = nc.dram_tensor("gt", [P, KT * T], BF16, kind="ExternalInput").ap()
    th = nc.dram_tensor("th", [P, NW], F32, kind="ExternalInput").ap()
    sc = nc.dram_tensor("sc", [P, 1], F32, kind="ExternalInput").ap()
    spk = nc.dram_tensor("spk", [NB_LOC * T, NW], BF16, kind="ExternalOutput").ap()

    with tile.TileContext(nc) as tc:
        with (
            tc.tile_pool(name="const", bufs=1) as cpool,
            tc.tile_pool(name="xin", bufs=2) as xpool,
            tc.tile_pool(name="xg", bufs=2) as xgpool,
            tc.tile_pool(name="sout", bufs=4) as spool,
            tc.tile_pool(name="ps1", bufs=2, space="PSUM") as ps1,
            tc.tile_pool(name="ps2", bufs=4, space="PSUM") as ps2,
        ):
            # spread input loads over independent DMA paths; stage-1
            # operands (gt, x0) first so PE starts ASAP
            gt_sb = cpool.tile([P, KT * T], BF16, tag="gt")
            nc.sync.dma_start(gt_sb, gt)
            x_sb = {}
            x_sb[0] = xpool.tile([P, KT * IN], BF16, tag="x", name="x0")
            nc.scalar.dma_start(x_sb[0], x[0:P, :])
            w_sb = cpool.tile([P, NCH, KP, 2, 512], FP8, tag="w")
            for j in range(NCH):
                nc.sync.dma_start(w_sb[:, j], w[:, j])
            th_sb = cpool.tile([P, NW], F32, tag="th")
            nc.scalar.dma_start(th_sb, th)
            sc_sb = cpool.tile([P, 1], F32, tag="sc")
            nc.sync.dma_start(sc_sb, sc)
            for b in range(1, NB_LOC):
                x_sb[b] = xpool.tile([P, KT * IN], BF16, tag="x", name=f"x{b}")
                nc.gpsimd.dma_start(x_sb[b], x[b * P : (b + 1) * P, :])

            for b in range(NB_LOC):
                # stage 1: xgT[i, t] = sum_s x_b[s, i] * GT[s, t]  (bf16)
                # GT[s, t] == 0 for t < s: s-tile k only feeds t >= 128k.
                xg_sb = [
                    xgpool.tile([P, 2, T], FP8, tag=f"xgp{kp}", name=f"xgp{kp}")
                    for kp in range(KP)
                ]
                for m in range(KI):
                    p1 = ps1.tile([P, T], F32, tag="p1")
                    for k in range(KT):
                        nc.tensor.matmul(
                            p1[:, k * P :],
                            x_sb[b][:, k * IN + m * P : k * IN + (m + 1) * P],
                            gt_sb[:, k * T + k * P : (k + 1) * T],
                            start=(k == 0),
                            stop=(k == KT - 1),
                            skip_group_check=True,
                        )
                    # PSUM->SBUF copy with runtime scale sx, cast to fp8
                    nc.scalar.activation(
                        xg_sb[m // 2][:, m % 2, :],
                        p1,
                        mybir.ActivationFunctionType.Copy,
                        scale=sc_sb[:, 0:1],
                    )
                # stage 2 (fp8 DoubleRow): mem[t, n] = sum_i xgT[i,t] W[i,n]
                for mt in range(KT):
                    s_sb = spool.tile([P, NW], BF16, tag="s")
                    for j in range(NCH):
                        p2 = ps2.tile([P, 512], F32, tag="p2")
                        for kp in range(KP):
                            nc.tensor.matmul(
                                p2,
                                xg_sb[kp][:, :, mt * P : (mt + 1) * P],
                                w_sb[:, j, kp],
                                start=(kp == 0),
                                stop=(kp == KP - 1),
                                perf_mode=mybir.MatmulPerfMode.DoubleRow,
                            )
                        nc.vector.tensor_tensor(
                            s_sb[:, j * 512 : (j + 1) * 512],
                            p2,
                            th_sb[:, j * 512 : (j + 1) * 512],
                            op=mybir.AluOpType.is_ge,
                        )
                    eng = nc.gpsimd if mt % 2 == 0 else nc.sync
                    eng.dma_start(
                        spk[b * T + mt * P : b * T + (mt + 1) * P, :], s_sb
                    )
    nc.compile()
    return nc


def _pow2_scale(target_max: float, value_max: float) -> float:
    """Largest power of two s with value_max * s <= target_max."""
    if value_max <= 0 or not np.isfinite(value_max):
        return 1.0
    return 2.0 ** math.floor(math.log2(target_max / value_max))


def _run_spmd_with_retry(nc, in_maps, trace=False, tries=3):
    """run_bass_kernel_spmd with retry: the first execution of a freshly
    compiled NEFF occasionally dies with a transient NRT error."""
    last = None
    for attempt in range(tries):
        try:
            return bass_utils.run_bass_kernel_spmd(
                nc, in_maps, core_ids=list(range(NCORES)), trace=trace
            )
        except Exception as e:  # noqa: BLE001
            last = e
    raise last


def _run_device(x_bm, W_in, gt_np, threshold, sx, sw, trace=False):
    """Run the SPMD kernel; returns (spikes [T,B,N] f32, results obj)."""
    global _compiled
    if _compiled is None:
        _compiled = _build_device()
    nc = _compiled
    x_bf = x_bm.astype(NPBF16)  # [B*T, IN]
    gt_bf = gt_np.astype(NPBF16)  # [T, T]
    gt_pack = np.ascontiguousarray(
        gt_bf.reshape(KT, P, T).transpose(1, 0, 2).reshape(P, KT * T)
    )
    x_pack_all = np.ascontiguousarray(
        x_bf.reshape(B, KT, P, IN).transpose(0, 2, 1, 3).reshape(B * P, KT * IN)
    )
    w_fp8 = (W_in.astype(np.float64) * sw).astype(np.float32).astype(NPFP8)
    th_scaled = (threshold.astype(np.float64) * (sx * sw)).astype(np.float32)
    sc_arr = np.full((P, 1), sx, dtype=np.float32)
    in_maps = []
    for c in range(NCORES):
        bg, ng = divmod(c, NNG)
        xs = np.ascontiguousarray(
            x_pack_all[bg * NB_LOC * P : (bg + 1) * NB_LOC * P]
        )
        # w[p, j, kp, i2, n] = W_in[(2kp+i2)*128+p, ng*NW + j*512+n] * sw
        wc = np.ascontiguousarray(
            w_fp8[:, ng * NW : (ng + 1) * NW]
            .reshape(KP, 2, P, NCH, 512)
            .transpose(2, 3, 0, 1, 4)
        )
        thc = np.ascontiguousarray(
            np.broadcast_to(th_scaled[ng * NW : (ng + 1) * NW], (P, NW))
        )
        in_maps.append(
            {"x": xs, "w": wc, "gt": gt_pack, "th": thc, "sc": sc_arr}
        )
    res = _run_spmd_with_retry(nc, in_maps, trace=trace)
    global LAST_RES
    LAST_RES = res
    out = np.zeros((B, T, N), dtype=np.float32)
    for c in range(NCORES):
        bg, ng = divmod(c, NNG)
        s = res.results[c]["spk"].astype(np.float32).reshape(NB_LOC, T, NW)
        out[bg * NB_LOC : (bg + 1) * NB_LOC, :, ng * NW : (ng + 1) * NW] = s
    return out.transpose(1, 0, 2), res


def _fallback(input_signal, weights, tau_mem, tau_syn, threshold):
    """Exact sequential port of the reference (numpy float32)."""
    x = np.asarray(input_signal, dtype=np.float32)
    w = np.asarray(weights, dtype=np.float32)
    W_in, W_rec = w[:IN], w[IN:]
    Tt, Bb, Nn = x.shape
    ff = np.einsum("tbi,in->tbn", x[:, :, :IN], W_in).astype(np.float32)
    syn = np.zeros((Bb, Nn), np.float32)
    mem = np.zeros((Bb, Nn), np.float32)
    fb = np.zeros((Bb, Nn), np.float32)
    out = np.zeros((Tt, Bb, Nn), np.float32)
    for t in range(Tt):
        cur = ff[t] + fb
        syn = syn + (-syn / tau_syn + cur) * np.float32(DT)
        mem = mem + (-mem / tau_mem + syn) * np.float32(DT)
        spikes = (mem >= threshold).astype(np.float32)
        mem = mem * (1.0 - spikes)
        rec = spikes[:, IN:] @ W_rec
        rec[:, :IN] = 0.0
        fb = rec
        out[t] = spikes
    return out


def kernel(input_signal, weights, tau_mem, tau_syn, threshold, _trace=False):
    input_signal = np.asarray(input_signal)
    weights = np.asarray(weights)
    tau_mem = np.asarray(tau_mem)
    tau_syn = np.asarray(tau_syn)
    threshold = np.asarray(threshold)

    ok_shape = (
        input_signal.shape == (T, B, N)
        and weights.shape == (N, N)
        and np.all(tau_mem == tau_mem.flat[0])
        and np.all(tau_syn == tau_syn.flat[0])
        and np.all(np.isfinite(input_signal))
        and np.all(np.isfinite(weights[:IN]))
        and np.all(np.isfinite(threshold))
    )
    if not ok_shape:
        return _fallback(input_signal, weights, tau_mem, tau_syn, threshold)

    alpha = 1.0 - DT / float(tau_syn.flat[0])
    beta = 1.0 - DT / float(tau_mem.flat[0])
    if not (0.0 <= alpha < 1.0 and 0.0 <= beta < 1.0):
        # numerically unstable / nonstandard regime: be safe
        return _fallback(input_signal, weights, tau_mem, tau_syn, threshold)

    gt_np = _build_gt(alpha, beta)

    # --- rigorous sub-threshold bound (exact arithmetic) -----------------
    # |mem[t,b,n]| <= max_i? no: mem = xg @ W with
    # |xg[i,t]| <= max_col||x_col||_2 * max_col||gt_col||_2
    # |mem[t,n]| <= ||xg[:,t]||_2 * ||W[:,n]||_2
    #            <= sum_d g(d)DT^2 * max_row||x_row||_2 * max_col||W_col||_2
    x_in = input_signal[:, :, :IN].astype(np.float64)
    W_in64 = weights[:IN].astype(np.float64)
    max_row = float(np.sqrt((x_in * x_in).sum(axis=2).max()))
    max_wcol = float(np.sqrt((W_in64 * W_in64).sum(axis=0).max()))
    gsum = float(_filter_taps(alpha, beta).sum())
    mem_bound = gsum * max_row * max_wcol

    # fp8 scale factors from data maxima / bounds (powers of two, exact)
    # xg bound: |xg[i,t]| <= max_i ||x[:,i]||_2 (per batch) * max_t ||gt[:,t]||_2
    xcol_max = float(
        np.sqrt(
            (x_in * x_in).sum(axis=0).max()  # sum over t for each (b, i)
        )
    )
    gtcol_max = float(np.sqrt((gt_np.astype(np.float64) ** 2).sum(axis=0).max()))
    xg_bound = xcol_max * gtcol_max
    w_max = float(np.abs(W_in64).max())
    sx = _pow2_scale(224.0, xg_bound)
    sw = _pow2_scale(224.0, w_max)

    # --- mixed-precision error allowance (conservative, absolute) -------
    # bf16 stage-1 relative error ~<1%; fp8 e4m3 operand rounding <=2^-4
    # relative each plus subnormal-flush floors eps = 2^-9/scale.
    eps_x = 2.0**-9 / sx
    eps_w = 2.0**-9 / sw
    err = (
        0.15 * mem_bound
        + IN * (eps_x * w_max + eps_w * xg_bound + eps_x * eps_w)
    )
    safe = (mem_bound + err) < float(threshold.min()) - MARGIN
    if not safe:
        return _fallback(input_signal, weights, tau_mem, tau_syn, threshold)

    # batch-major rows: row (b*T + t) = input_signal[t, b, :IN]
    x_bm = np.ascontiguousarray(
        input_signal[:, :, :IN].transpose(1, 0, 2).reshape(B * T, IN)
    ).astype(np.float32, copy=False)
    W_in = np.ascontiguousarray(weights[:IN]).astype(np.float32, copy=False)

    spikes, _ = _run_device(
        x_bm, W_in, gt_np, threshold.astype(np.float32), sx, sw, trace=_trace
    )
    if spikes.any():
        # bound said sub-threshold yet device saw spikes: distrust, recompute
        return _fallback(input_signal, weights, tau_mem, tau_syn, threshold)
    return spikes


# revision 18
# speedup vs baseline: 1.1367x; 1.0129x over previous
"""Trainium2 Bass kernel for nn_EvolvableSNN (T=512, B=8, N=4096, LIF SNN).

Strategy
--------
The LIF dynamics with these parameters are sub-threshold: the membrane
potential equilibrium is ~tau_mem*tau_syn*cur ~= 1e-4 * cur, four orders of
magnitude below threshold=1.0, so no neuron ever spikes and the recurrent
feedback term is identically zero.  With zero feedback the scan is a LINEAR
time-invariant filter of the feedforward drive:

    ff    = input[:, :, :512] @ W_in                      # [T, B, N]
    mem_t = DT^2 * sum_{s<=t} g(t-s) * ff_s               # per (b, n)
    g(d)  = (b^(d+1) - a^(d+1)) / (b - a),  a = 1-DT/tau_syn, b = 1-DT/tau_mem
    spikes_t = (mem_t >= threshold)

so mem = GT.T @_time (x @ W_in) -- two chained dense matmuls, fully parallel
across (batch, neuron).  Validity is guarded by a rigorous norm bound
computed on the host:

    max|mem| <= DT^2 * sum_d g(d) * max_row||x_row||_2 * max_col||W_col||_2

(~2e-3 for the target inputs, vs threshold 1.0).  If the bound (inflated by
the mixed-precision error allowance, see below) does not clear
min(threshold) by a wide margin -- or the device reports any spike -- we
fall back to an exact sequential numpy port of the reference.  The first
spike of the no-feedback system coincides with the first spike of the true
system, so "no spikes under linearization" exactly implies correctness.

Numerics: stage 1 (time filter) runs in bf16 operands with fp32 PSUM
accumulation; stage 2 (x W_in product) runs in fp8-e4m3 DoubleRow (2x PE
throughput) with power-of-two scale factors sx (on xg, applied by the
Scalar-engine PSUM->SBUF copy) and sw (folded into W on the host).  The
threshold is pre-scaled by sx*sw on the host, so the comparison
(mem*sx*sw >= th*sx*sw) is exactly monotone-equivalent.  Spike values {0,1}
are exact in the bf16 output, which the host casts back to fp32.

Sharding: (NBG batch-groups x NNG neuron-column-groups) grid over 8 cores.
Each core runs the same program on its input slice; no collectives.
"""

import math

import numpy as np
import ml_dtypes

import concourse.bass as bass
import concourse.mybir as mybir
import concourse.tile as tile
from concourse import bacc, bass_utils

# Problem constants (hardcoded per harness contract).
T, B, N = 512, 8, 4096
IN = 512          # INPUT_SIZE
DT = 0.001
P = 128           # SBUF partitions
NCORES = 8

# Core grid: NBG batch-groups x NNG neuron-groups (NBG * NNG == NCORES).
NBG, NNG = 4, 2
NB_LOC = B // NBG          # batches per core
NW = N // NNG              # neuron columns per core
KI = IN // P               # contraction tiles over input dim (4)
KP = KI // 2               # DoubleRow contraction pair-tiles (2)
KT = T // P                # tiles over time dim (4)
NCH = NW // 512            # 512-wide n chunks per core
F32 = mybir.dt.float32
BF16 = mybir.dt.bfloat16
FP8 = mybir.dt.float8e4
NPBF16 = ml_dtypes.bfloat16
NPFP8 = ml_dtypes.float8_e4m3

MARGIN = 0.1               # abs margin to min(threshold) for the fast path

_compiled = None           # cached compiled Bass module
LAST_RES = None            # last device results (for external profiling)


def _filter_taps(alpha: float, beta: float) -> np.ndarray:
    """g(d) * DT^2 for d = 0..T-1 (float64)."""
    d = np.arange(T, dtype=np.float64)
    if abs(beta - alpha) > 1e-12:
        g = (beta ** (d + 1) - alpha ** (d + 1)) / (beta - alpha)
    else:
        g = (d + 1) * alpha**d
    return g * DT * DT


def _build_gt(alpha: float, beta: float) -> np.ndarray:
    """GT[s, t] = DT^2 * g(t - s) for s <= t else 0 (upper-triangular)."""
    g = _filter_taps(alpha, beta)
    s = np.arange(T)
    diff = s[None, :] - s[:, None]  # diff[s, t] = t - s
    gt = np.where(diff >= 0, g[np.clip(diff, 0, T - 1)], 0.0)
    return gt.astype(np.float32)


def _build_device():
    """Compile the per-core Tile kernel once; returns the Bass module.

    Input layouts are pre-packed on the host so every DMA is one large
    fully-contiguous transfer:
      x  [NB_LOC*P, KT*IN]     row (b*P + p), col (k*IN + i) = x_b[k*P+p, i]
      w  [P, NCH, KP, 2, 512]  fp8, w[p, j, kp, i2, n]
                               = W_in[(2kp+i2)*128+p, j*512+n] * sw
      gt [P, KT*T]             row p, col (k*T + t) = GT[k*P+p, t]
      th [P, NW]               threshold * sx * sw, replicated rows
      sc [P, 1]                sx (runtime scale for the stage-1 copy)
    """
    nc = bacc.Bacc(
        "TRN2", target_bir_lowering=False, debug=False, num_devices=NCORES
    )
    x = nc.dram_tensor("x", [NB_LOC * P, KT * IN], BF16, kind="ExternalInput").ap()
    w = nc.dram_tensor("w", [P, NCH, KP, 2, 512], FP8, kind="ExternalInput").ap()
    gt = nc.dram_tensor("gt", [P, KT * T], BF16, kind="ExternalInput").ap()
    th = nc.dram_tensor("th", [P, NW], F32, kind="ExternalInput").ap()
    sc = nc.dram_tensor("sc", [P, 1], F32, kind="ExternalInput").ap()
    spk = nc.dram_tensor("spk", [NB_LOC * T, NW], FP8, kind="ExternalOutput").ap()

    with tile.TileContext(nc) as tc:
        with (
            tc.tile_pool(name="const", bufs=1) as cpool,
            tc.tile_pool(name="xin", bufs=2) as xpool,
            tc.tile_pool(name="xg", bufs=2) as xgpool,
            tc.tile_pool(name="sout", bufs=4) as spool,
            tc.tile_pool(name="ps1", bufs=2, space="PSUM") as ps1,
            tc.tile_pool(name="ps2", bufs=3, space="PSUM") as ps2,
        ):
            # spread input loads over independent DMA paths; stage-1
            # operands (gt, x0) first so PE starts ASAP
            gt_sb = cpool.tile([P, KT * T], BF16, tag="gt")
            nc.sync.dma_start(gt_sb, gt)
            x_sb = {}
            x_sb[0] = xpool.tile([P, KT * IN], BF16, tag="x", name="x0")
            nc.scalar.dma_start(x_sb[0], x[0:P, :])
            w_sb = cpool.tile([P, NCH, KP, 2, 512], FP8, tag="w")
            for j in range(NCH):
                nc.sync.dma_start(w_sb[:, j], w[:, j])
            th_sb = cpool.tile([P, NW], F32, tag="th")
            nc.scalar.dma_start(th_sb, th)
            sc_sb = cpool.tile([P, 1], F32, tag="sc")
            nc.sync.dma_start(sc_sb, sc)
            for b in range(1, NB_LOC):
                x_sb[b] = xpool.tile([P, KT * IN], BF16, tag="x", name=f"x{b}")
                nc.gpsimd.dma_start(x_sb[b], x[b * P : (b + 1) * P, :])

            for b in range(NB_LOC):
                # stage 1: xgT[i, t] = sum_s x_b[s, i] * GT[s, t]  (bf16)
                # GT[s, t] == 0 for t < s: s-tile k only feeds t >= 128k.
                xg_sb = [
                    xgpool.tile([P, 2, T], FP8, tag=f"xgp{kp}", name=f"xgp{kp}")
                    for kp in range(KP)
                ]
                for m in range(KI):
                    p1 = ps1.tile([P, T], F32, tag="p1")
                    for k in range(KT):
                        nc.tensor.matmul(
                            p1[:, k * P :],
                            x_sb[b][:, k * IN + m * P : k * IN + (m + 1) * P],
                            gt_sb[:, k * T + k * P : (k + 1) * T],
                            start=(k == 0),
                            stop=(k == KT - 1),
                            skip_group_check=True,
                        )
                    # PSUM->SBUF copy with runtime scale sx, cast to fp8
                    nc.scalar.activation(
                        xg_sb[m // 2][:, m % 2, :],
                        p1,
                        mybir.ActivationFunctionType.Copy,
                        scale=sc_sb[:, 0:1],
                    )
                # stage 2 (fp8 DoubleRow): mem[t, n] = sum_i xgT[i,t] W[i,n]
                # PSUM tiles are 1024 wide (2 matmul groups) so the is_ge
                # compare amortizes the DVE fixed cost over 1024 columns.
                for mt in range(KT):
                    s_sb = spool.tile([P, NW], FP8, tag="s")
                    for j2 in range(NCH // 2):
                        p2 = ps2.tile([P, 1024], F32, tag="p2")
                        for jh in range(2):
                            j = 2 * j2 + jh
                            for kp in range(KP):
                                nc.tensor.matmul(
                                    p2[:, jh * 512 : (jh + 1) * 512],
                                    xg_sb[kp][:, :, mt * P : (mt + 1) * P],
                                    w_sb[:, j, kp],
                                    start=(kp == 0),
                                    stop=(kp == KP - 1),
                                    perf_mode=mybir.MatmulPerfMode.DoubleRow,
                                    skip_group_check=True,
                                )
                        nc.vector.tensor_tensor(
                            s_sb[:, j2 * 1024 : (j2 + 1) * 1024],
                            p2,
                            th_sb[:, j2 * 1024 : (j2 + 1) * 1024],
                            op=mybir.AluOpType.is_ge,
                        )
                    eng = nc.gpsimd if mt % 2 == 0 else nc.sync
                    eng.dma_start(
                        spk[b * T + mt * P : b * T + (mt + 1) * P, :], s_sb
                    )
    nc.compile()
    return nc


def _pow2_scale(target_max: float, value_max: float) -> float:
    """Largest power of two s with value_max * s <= target_max."""
    if value_max <= 0 or not np.isfinite(value_max):
        return 1.0
    return 2.0 ** math.floor(math.log2(target_max / value_max))


def _run_spmd_with_retry(nc, in_maps, trace=False, tries=3):
    """run_bass_kernel_spmd with retry: the first execution of a freshly
    compiled NEFF occasionally dies with a transient NRT error."""
    last = None
    for attempt in range(tries):
        try:
            return bass_utils.run_bass_kernel_spmd(
                nc, in_maps, core_ids=list(range(NCORES)), trace=trace
            )
        except Exception as e:  # noqa: BLE001
            last = e
    raise last


def _run_device(x_bm, W_in, gt_np, threshold, sx, sw, trace=False):
    """Run the SPMD kernel; returns (spikes [T,B,N] f32, results obj)."""
    global _compiled
    if _compiled is None:
        _compiled = _build_device()
    nc = _compiled
    x_bf = x_bm.astype(NPBF16)  # [B*T, IN]
    gt_bf = gt_np.astype(NPBF16)  # [T, T]
    gt_pack = np.ascontiguousarray(
        gt_bf.reshape(KT, P, T).transpose(1, 0, 2).reshape(P, KT * T)
    )
    x_pack_all = np.ascontiguousarray(
        x_bf.reshape(B, KT, P, IN).transpose(0, 2, 1, 3).reshape(B * P, KT * IN)
    )
    w_fp8 = (W_in.astype(np.float64) * sw).astype(np.float32).astype(NPFP8)
    th_scaled = (threshold.astype(np.float64) * (sx * sw)).astype(np.float32)
    sc_arr = np.full((P, 1), sx, dtype=np.float32)
    in_maps = []
    for c in range(NCORES):
        bg, ng = divmod(c, NNG)
        xs = np.ascontiguousarray(
            x_pack_all[bg * NB_LOC * P : (bg + 1) * NB_LOC * P]
        )
        # w[p, j, kp, i2, n] = W_in[(2kp+i2)*128+p, ng*NW + j*512+n] * sw
        wc = np.ascontiguousarray(
            w_fp8[:, ng * NW : (ng + 1) * NW]
            .reshape(KP, 2, P, NCH, 512)
            .transpose(2, 3, 0, 1, 4)
        )
        thc = np.ascontiguousarray(
            np.broadcast_to(th_scaled[ng * NW : (ng + 1) * NW], (P, NW))
        )
        in_maps.append(
            {"x": xs, "w": wc, "gt": gt_pack, "th": thc, "sc": sc_arr}
        )
    res = _run_spmd_with_retry(nc, in_maps, trace=trace)
    global LAST_RES
    LAST_RES = res
    out = np.zeros((B, T, N), dtype=np.float32)
    for c in range(NCORES):
        bg, ng = divmod(c, NNG)
        s = res.results[c]["spk"].astype(np.float32).reshape(NB_LOC, T, NW)
        out[bg * NB_LOC : (bg + 1) * NB_LOC, :, ng * NW : (ng + 1) * NW] = s
    return out.transpose(1, 0, 2), res


def _fallback(input_signal, weights, tau_mem, tau_syn, threshold):
    """Exact sequential port of the reference (numpy float32)."""
    x = np.asarray(input_signal, dtype=np.float32)
    w = np.asarray(weights, dtype=np.float32)
    W_in, W_rec = w[:IN], w[IN:]
    Tt, Bb, Nn = x.shape
    ff = np.einsum("tbi,in->tbn", x[:, :, :IN], W_in).astype(np.float32)
    syn = np.zeros((Bb, Nn), np.float32)
    mem = np.zeros((Bb, Nn), np.float32)
    fb = np.zeros((Bb, Nn), np.float32)
    out = np.zeros((Tt, Bb, Nn), np.float32)
    for t in range(Tt):
        cur = ff[t] + fb
        syn = syn + (-syn / tau_syn + cur) * np.float32(DT)
        mem = mem + (-mem / tau_mem + syn) * np.float32(DT)
        spikes = (mem >= threshold).astype(np.float32)
        mem = mem * (1.0 - spikes)
        rec = spikes[:, IN:] @ W_rec
        rec[:, :IN] = 0.0
        fb = rec
        out[t] = spikes
    return out


def kernel(input_signal, weights, tau_mem, tau_syn, threshold, _trace=False):
    input_signal = np.asarray(input_signal)
    weights = np.asarray(weights)
    tau_mem = np.asarray(tau_mem)
    tau_syn = np.asarray(tau_syn)
    threshold = np.asarray(threshold)

    ok_shape = (
        input_signal.shape == (T, B, N)
        and weights.shape == (N, N)
        and np.all(tau_mem == tau_mem.flat[0])
        and np.all(tau_syn == tau_syn.flat[0])
        and np.all(np.isfinite(input_signal))
        and np.all(np.isfinite(weights[:IN]))
        and np.all(np.isfinite(threshold))
    )
    if not ok_shape:
        return _fallback(input_signal, weights, tau_mem, tau_syn, threshold)

    alpha = 1.0 - DT / float(tau_syn.flat[0])
    beta = 1.0 - DT / float(tau_mem.flat[0])
    if not (0.0 <= alpha < 1.0 and 0.0 <= beta < 1.0):
        # numerically unstable / nonstandard regime: be safe
        return _fallback(input_signal, weights, tau_mem, tau_syn, threshold)

    gt_np = _build_gt(alpha, beta)

    # --- rigorous sub-threshold bound (exact arithmetic) -----------------
    # |mem[t,b,n]| <= max_i? no: mem = xg @ W with
    # |xg[i,t]| <= max_col||x_col||_2 * max_col||gt_col||_2
    # |mem[t,n]| <= ||xg[:,t]||_2 * ||W[:,n]||_2
    #            <= sum_d g(d)DT^2 * max_row||x_row||_2 * max_col||W_col||_2
    x_in = input_signal[:, :, :IN].astype(np.float64)
    W_in64 = weights[:IN].astype(np.float64)
    max_row = float(np.sqrt((x_in * x_in).sum(axis=2).max()))
    max_wcol = float(np.sqrt((W_in64 * W_in64).sum(axis=0).max()))
    gsum = float(_filter_taps(alpha, beta).sum())
    mem_bound = gsum * max_row * max_wcol

    # fp8 scale factors from data maxima / bounds (powers of two, exact)
    # xg bound: |xg[i,t]| <= max_i ||x[:,i]||_2 (per batch) * max_t ||gt[:,t]||_2
    xcol_max = float(
        np.sqrt(
            (x_in * x_in).sum(axis=0).max()  # sum over t for each (b, i)
        )
    )
    gtcol_max = float(np.sqrt((gt_np.astype(np.float64) ** 2).sum(axis=0).max()))
    xg_bound = xcol_max * gtcol_max
    w_max = float(np.abs(W_in64).max())
    sx = _pow2_scale(224.0, xg_bound)
    sw = _pow2_scale(224.0, w_max)

    # --- mixed-precision error allowance (conservative, absolute) -------
    # bf16 stage-1 relative error ~<1%; fp8 e4m3 operand rounding <=2^-4
    # relative each plus subnormal-flush floors eps = 2^-9/scale.
    eps_x = 2.0**-9 / sx
    eps_w = 2.0**-9 / sw
    err = (
        0.15 * mem_bound
        + IN * (eps_x * w_max + eps_w * xg_bound + eps_x * eps_w)
    )
    safe = (mem_bound + err) < float(threshold.min()) - MARGIN
    if not safe:
        return _fallback(input_signal, weights, tau_mem, tau_syn, threshold)

    # batch-major rows: row (b*T + t) = input_signal[t, b, :IN]
    x_bm = np.ascontiguousarray(
        input_signal[:, :, :IN].transpose(1, 0, 2).reshape(B * T, IN)
    ).astype(np.float32, copy=False)
    W_in = np.ascontiguousarray(weights[:IN]).astype(np.float32, copy=False)

    spikes, _ = _run_device(
        x_bm, W_in, gt_np, threshold.astype(np.float32), sx, sw, trace=_trace
    )
    if spikes.any():
        # bound said sub-threshold yet device saw spikes: distrust, recompute
        return _fallback(input_signal, weights, tau_mem, tau_syn, threshold)
    return spikes
